# revision 36
# baseline (speedup 1.0000x reference)
"""Trainium2 Bass kernel for the 14-wire quantum autoencoder swap test.

Math: wires 10-13 stay |0> until the swap test, so
P(aux=1) = 0.5 - 0.5*q with q = sum_{trash wires 7,8,9 = 0} |c_i|^2 of the
10-qubit state after AngleEmbedding + BasicEntanglerLayers.

All transforms run on the PE in fp16 (fp32 PSUM accumulation), data-parallel
over 8 cores (32 samples each), two 16-sample half-batch pipelines per core:
  state S~ [ft, w] per half, ft = par*64 + w0*32 + w1*16 + bl
  (par = w2 after C01,C12; w = wires 3..9 index, w9 = MSB).
  Per layer: stage G (RX0-2 + C01 + C12; C90 of the previous layer folded in
  via row-permuted GkC90 blocks applied to the w9=1 column half) flips to
  standard layout [w, ft']; stage K (RX3-9 + C34..C89, C23 via K2/K2b
  stationary split by par class) flips back. Zero-padded stationary columns
  keep every matmul writing all 128 PSUM partitions. Negations live in
  host-built blocks: [Xim_neg | Xre | Xim] per matrix. PSUM->SBUF copies are
  split re(ACT)/im(DVE) and the per-layer emission interleaves both halves
  so the PE never stalls on one half's copies.
The embedding is folded into layer 0's G stage: S~0 = diag(F) @ P with P the
per-sample wires-3..9 seed (host-replicated over g classes) and GF =
diag(F) @ Gk0 host-premultiplied. Layer 3's K stage computes only the 16
w<16 output columns the projection needs; the finale is an ACT
square+row-accumulate, a (-0.5*sel)-matmul with +0.5 bias row, and one DMA
per half.
"""
import numpy as np

NCORES = 8
B_CORE = 32
HB = 16
DEPTH = 4
NQ = 10

# pk16 fp16 [128, 1728]: P per half (2x384, [imn|re|im]) |
# GF = diag(F)*Gk0 per half (2x384, [imn|re|im]) | zeros 192
C_P = 0        # + hb*640: [Pre 128 | Pim 128]
C_GF = 256     # + hb*640: [GFimn | GFre | GFim]
C_ZERO = 1280
PK16 = 1472

# pk32 f32 [128, 34]: cols 0:16 = -0.5*sel, col 16 = 1.0, cols 18:34 = 0.5
PK32 = 34

# mats fp16: l0: kR 384 | kB 384; l1/l2: gR | gC | kR | kB;
# l3: gR | gC | kR16 (48) | kB16 (48)
L_COLS = [768, 1536, 1536, 864]
L_OFF = [0, 768, 2304, 3840]
M_COLS = 4704

# ---------------------------------------------------------------------------
# Host-side constant construction
# ---------------------------------------------------------------------------

# ft class (par*4 + gg) -> g = w0*4 + w1*2 + w2
_FT_G = np.zeros(8, dtype=np.int64)
for _par in range(2):
    for _gg in range(4):
        _FT_G[_par * 4 + _gg] = (_gg >> 1) * 4 + (_gg & 1) * 2 + _par


def _perm_matrix(perm):
    m = np.zeros((len(perm), len(perm)))
    for src, dst in enumerate(perm):
        m[dst, src] = 1.0
    return m


def _cnot_chain_perm_p():
    perm = np.zeros(128, dtype=np.int64)
    for p in range(128):
        w = [(p >> k) & 1 for k in range(7)]
        for k in range(6):
            w[k + 1] ^= w[k]
        perm[p] = sum(w[k] << k for k in range(7))
    return perm


def _build_k2(weights_l):
    m = np.array([[1.0]], dtype=np.complex128)
    for w in (9, 8, 7, 6, 5, 4, 3):
        c, s = np.cos(weights_l[w] / 2), np.sin(weights_l[w] / 2)
        r = np.array([[c, -1j * s], [-1j * s, c]], dtype=np.complex128)
        m = np.kron(m, r)
    qa = _perm_matrix(_cnot_chain_perm_p())
    k2 = qa @ m
    k2b = k2 @ _perm_matrix(np.arange(128) ^ 1)
    return k2, k2b


def _build_gk(weights_l):
    m = np.array([[1.0]], dtype=np.complex128)
    for w in (0, 1, 2):
        c, s = np.cos(weights_l[w] / 2), np.sin(weights_l[w] / 2)
        r = np.array([[c, -1j * s], [-1j * s, c]], dtype=np.complex128)
        m = np.kron(m, r)
    perm = np.zeros(8, dtype=np.int64)
    for g in range(8):
        w0, w1, w2 = (g >> 2) & 1, (g >> 1) & 1, g & 1
        w1 ^= w0
        w2 ^= w1
        perm[g] = w0 * 4 + w1 * 2 + w2
    G = _perm_matrix(perm) @ m
    gk = np.zeros((128, 128), dtype=np.complex128)
    for ci in range(8):
        for co in range(8):
            v = G[_FT_G[co], _FT_G[ci]]
            if v != 0:
                for bl in range(HB):
                    gk[ci * 16 + bl, co * 16 + bl] = v
    return gk  # contraction: T[ft_out] = sum_ft_in S[ft_in] * gk[ft_in, ft_out]


def _blocks(m):
    """[im_neg | re | im] fp column triple for complex matrix m [128,128]."""
    return np.concatenate([-m.imag, m.real, m.imag], axis=1)


def _make_mats(weights):
    wt = weights.astype(np.float64).reshape(DEPTH, NQ)
    flip = np.arange(128) ^ 32
    cols = []
    for l in range(DEPTH):
        k2, k2b = _build_k2(wt[l])
        if l >= 1:
            gk = _build_gk(wt[l])
            cols.append(_blocks(gk))
            cols.append(_blocks(gk[flip, :]))
        if l < DEPTH - 1:
            cols.append(_blocks(k2.T))
            cols.append(_blocks(k2b.T))
        else:
            cols.append(_blocks(k2.T[:, 0:16]))
            cols.append(_blocks(k2b.T[:, 0:16]))
    mats = np.concatenate(cols, axis=1)
    assert mats.shape == (128, M_COLS)
    return mats.astype(np.float16)


def _make_pk16(features_core, weights_l0):
    th = features_core.astype(np.float64)
    B = th.shape[0]
    c_emb, s_emb = np.cos(th / 2), np.sin(th / 2)
    v = np.stack([c_emb.astype(np.complex128), -1j * s_emb], axis=-1)

    pk = np.zeros((128, PK16), dtype=np.float64)
    # seed over wires 3..9 per sample: pt[b, j], j = w9*64+...+w3
    pt = np.ones((B, 128), dtype=np.complex128)
    for j in range(128):
        val = np.ones(B, dtype=np.complex128)
        for k, w in enumerate((3, 4, 5, 6, 7, 8, 9)):
            val = val * v[:, w, (j >> k) & 1]
        pt[:, j] = val
    # F per sample per class
    Fv = np.zeros((B, 8), dtype=np.complex128)
    for cls in range(8):
        g = _FT_G[cls]
        w0, w1, w2 = (g >> 2) & 1, (g >> 1) & 1, g & 1
        Fv[:, cls] = v[:, 0, w0] * v[:, 1, w1] * v[:, 2, w2]
    gk0 = _build_gk(weights_l0)
    for hb in range(2):
        # P rows: ft = cls*16+bl -> pt[hb*16+bl]; GF = diag(F)*gk0
        P = np.zeros((128, 128), dtype=np.complex128)
        GF = np.zeros((128, 128), dtype=np.complex128)
        for cls in range(8):
            for bl in range(HB):
                ft = cls * 16 + bl
                b = hb * HB + bl
                P[ft, :] = pt[b, :]
                GF[ft, :] = Fv[b, cls] * gk0[ft, :]
        c0 = C_P + hb * 640
        pk[:, c0:c0 + 128] = P.real
        pk[:, c0 + 128:c0 + 256] = P.imag
        c1 = C_GF + hb * 640
        pk[:, c1:c1 + 128] = -GF.imag
        pk[:, c1 + 128:c1 + 256] = GF.real
        pk[:, c1 + 256:c1 + 384] = GF.imag
    return pk.astype(np.float16)


def _make_pk32():
    pk = np.zeros((128, PK32), dtype=np.float32)
    for ft in range(128):
        pk[ft, ft % 16] = -0.5
    pk[:, 16] = 1.0
    pk[:, 18:34] = 0.5
    return pk


# ---------------------------------------------------------------------------
# Bass program
# ---------------------------------------------------------------------------

_PROGRAM = None


def _build_program():
    import concourse.bacc as bacc
    import concourse.mybir as mybir
    import concourse.tile as tile

    F32 = mybir.dt.float32
    F16 = mybir.dt.float16
    MULT = mybir.AluOpType.mult
    ADD = mybir.AluOpType.add

    nc = bacc.Bacc("TRN2", target_bir_lowering=False, debug=False,
                   num_devices=NCORES)

    d_pk16 = nc.dram_tensor("pk16", [128, PK16], F16, kind="ExternalInput")
    d_pk32 = nc.dram_tensor("pk32", [128, PK32], F32, kind="ExternalInput")
    d_mats = nc.dram_tensor("mats", [128, M_COLS], F16, kind="ExternalInput")
    d_out = nc.dram_tensor("out", [1, B_CORE], F32, kind="ExternalOutput")

    with tile.TileContext(nc) as tc:
        with (
            tc.tile_pool(name="const", bufs=1) as cpool,
            tc.tile_pool(name="psum", bufs=8, space="PSUM") as ppool,
        ):
            t_pk16 = cpool.tile([128, PK16], F16, tag="pk16")
            t_pk32 = cpool.tile([128, PK32], F32, tag="pk32")
            t_mats = cpool.tile([128, M_COLS], F16, tag="mats")

            # DMAs in just-in-time order, single SP queue
            nc.sync.dma_start(t_pk16[:, 0:C_ZERO], d_pk16[:, 0:C_ZERO])
            nc.sync.dma_start(t_pk16[:, C_ZERO:PK16], d_pk16[:, C_ZERO:PK16])
            nc.sync.dma_start(t_mats[:, 0:768], d_mats[:, 0:768])
            nc.sync.dma_start(t_mats[:, 768:1536], d_mats[:, 768:1536])
            nc.sync.dma_start(t_mats[:, 1536:2304], d_mats[:, 1536:2304])
            for l in range(2, DEPTH):
                sl = slice(L_OFF[l], L_OFF[l] + L_COLS[l])
                nc.sync.dma_start(t_mats[:, sl], d_mats[:, sl])
            nc.sync.dma_start(t_pk32[:], d_pk32[:])

            # PE warm-up: starts the PE ramp clock early
            t_wu = cpool.tile([128, 32], F16, tag="wu")
            nc.gpsimd.memset(t_wu[:], 0.0)
            ps_wu = ppool.tile([32, 32], F32, tag="ps", name="wu")
            for i in range(3):
                nc.tensor.matmul(ps_wu[:], t_wu[:], t_wu[:],
                                 start=True, stop=True)

            # state tiles (persistent; zero-padded layout [128, 512]:
            # data chunks at 0,128,256,384 (64 cols), Z at 64,192,320)
            def zfill(t, eng):
                dst = t[:].rearrange("p (a b) -> p a b", a=4, b=128)
                src = t_pk16[:, C_ZERO:C_ZERO + 192].rearrange(
                    "p (a b) -> p a b", a=3, b=64)
                eng(dst[:, 0:3, 64:128], src)

            sAB = []
            tt = []
            for hb in range(2):
                a = cpool.tile([128, 512], F16, tag=f"sA{hb}", name=f"sA{hb}")
                b = cpool.tile([128, 512], F16, tag=f"sB{hb}", name=f"sB{hb}")
                t = cpool.tile([128, 512], F16, tag=f"tt{hb}", name=f"tt{hb}")
                sAB.append([a, b])
                tt.append(t)
            for hb in range(2):
                zfill(tt[hb], nc.vector.tensor_copy)
            for hb in range(2):
                zfill(sAB[hb][0], nc.vector.tensor_copy)
            for hb in range(2):
                zfill(sAB[hb][1], nc.gpsimd.tensor_copy)

            # chunk views of a zero-padded tile: [p, x(par/w9), y(ri), 64]
            def chunks(t):
                return t[:].rearrange("p (x y b) -> p x y b",
                                      x=2, y=2, b=128)[:, :, :, 0:64]

            # ---------------- layers ----------------
            def mat(c0, c1):
                return t_mats[:, c0:c1]

            psK3 = [None, None]

            def emit_g(l, hb, gR, gC):
                pgr = ppool.tile([128, 128], F32, tag="ps",
                                 name=f"pgr{l}{hb}")
                pgi = ppool.tile([128, 128], F32, tag="ps",
                                 name=f"pgi{l}{hb}")
                if l == 0:
                    c0 = C_P + hb * 640
                    c1 = C_GF + hb * 640
                    pre = t_pk16[:, c0:c0 + 128]
                    pim = t_pk16[:, c0 + 128:c0 + 256]
                    gfimn = t_pk16[:, c1:c1 + 128]
                    gfre = t_pk16[:, c1 + 128:c1 + 256]
                    gfim = t_pk16[:, c1 + 256:c1 + 384]
                    nc.tensor.matmul(pgr[:], pre, gfre,
                                     start=True, stop=False)
                    nc.tensor.matmul(pgr[:], pim, gfimn,
                                     start=False, stop=True)
                    nc.tensor.matmul(pgi[:], pre, gfim,
                                     start=True, stop=False)
                    nc.tensor.matmul(pgi[:], pim, gfre,
                                     start=False, stop=True)
                else:
                    sv = sAB[hb][(l - 1) % 2]
                    nc.tensor.matmul(pgr[:], sv[:, 0:128],
                                     mat(gR + 128, gR + 256),
                                     start=True, stop=False)
                    nc.tensor.matmul(pgr[:], sv[:, 192:320],
                                     mat(gC + 128, gC + 256),
                                     start=False, stop=False)
                    nc.tensor.matmul(pgr[:], sv[:, 128:256],
                                     mat(gR, gR + 128),
                                     start=False, stop=False)
                    nc.tensor.matmul(pgr[:], sv[:, 320:448],
                                     mat(gC, gC + 128),
                                     start=False, stop=True)
                    nc.tensor.matmul(pgi[:], sv[:, 0:128],
                                     mat(gR + 256, gR + 384),
                                     start=True, stop=False)
                    nc.tensor.matmul(pgi[:], sv[:, 192:320],
                                     mat(gC + 256, gC + 384),
                                     start=False, stop=False)
                    nc.tensor.matmul(pgi[:], sv[:, 128:256],
                                     mat(gR + 128, gR + 256),
                                     start=False, stop=False)
                    nc.tensor.matmul(pgi[:], sv[:, 320:448],
                                     mat(gC + 128, gC + 256),
                                     start=False, stop=True)
                return pgr, pgi

            def emit_k(l, hb, kR, kB):
                t = tt[hb]
                if l < DEPTH - 1:
                    pkr = ppool.tile([128, 128], F32, tag="ps",
                                     name=f"pkr{l}{hb}")
                    pki = ppool.tile([128, 128], F32, tag="ps",
                                     name=f"pki{l}{hb}")
                    nc.tensor.matmul(pkr[:], t[:, 0:128],
                                     mat(kR + 128, kR + 256),
                                     start=True, stop=False)
                    nc.tensor.matmul(pkr[:], t[:, 192:320],
                                     mat(kB + 128, kB + 256),
                                     start=False, stop=False)
                    nc.tensor.matmul(pkr[:], t[:, 128:256],
                                     mat(kR, kR + 128),
                                     start=False, stop=False)
                    nc.tensor.matmul(pkr[:], t[:, 320:448],
                                     mat(kB, kB + 128),
                                     start=False, stop=True)
                    nc.tensor.matmul(pki[:], t[:, 0:128],
                                     mat(kR + 256, kR + 384),
                                     start=True, stop=False)
                    nc.tensor.matmul(pki[:], t[:, 192:320],
                                     mat(kB + 256, kB + 384),
                                     start=False, stop=False)
                    nc.tensor.matmul(pki[:], t[:, 128:256],
                                     mat(kR + 128, kR + 256),
                                     start=False, stop=False)
                    nc.tensor.matmul(pki[:], t[:, 320:448],
                                     mat(kB + 128, kB + 256),
                                     start=False, stop=True)
                    return pkr, pki
                psK3[hb] = ppool.tile([128, 32], F32, tag="ps",
                                      name=f"pk3{hb}")
                nc.tensor.matmul(psK3[hb][:], t[:, 0:128],
                                 mat(kR + 16, kR + 48),
                                 start=True, stop=False)
                nc.tensor.matmul(psK3[hb][:], t[:, 192:320],
                                 mat(kB + 16, kB + 48),
                                 start=False, stop=False)
                nc.tensor.matmul(psK3[hb][:], t[:, 128:256],
                                 mat(kR, kR + 32),
                                 start=False, stop=False)
                nc.tensor.matmul(psK3[hb][:], t[:, 320:448],
                                 mat(kB, kB + 32),
                                 start=False, stop=True)
                return None

            for l in range(DEPTH):
                base = L_OFF[l]
                gR = base
                gC = base + 384
                kR = base + (768 if l >= 1 else 0)
                kB = kR + (48 if l == DEPTH - 1 else 384)
                pg = [emit_g(l, hb, gR, gC) for hb in range(2)]
                for hb in range(2):
                    tch = chunks(tt[hb])
                    nc.scalar.copy(
                        tch[:, :, 0],
                        pg[hb][0][:].rearrange("p (par c) -> p par c", par=2))
                    nc.vector.tensor_copy(
                        tch[:, :, 1],
                        pg[hb][1][:].rearrange("p (par c) -> p par c", par=2))
                pk = [emit_k(l, hb, kR, kB) for hb in range(2)]
                if l < DEPTH - 1:
                    for hb in range(2):
                        sch = chunks(sAB[hb][l % 2])
                        nc.vector.tensor_copy(
                            sch[:, :, 0],
                            pk[hb][0][:].rearrange("p (w9 c) -> p w9 c",
                                                   w9=2))
                        nc.scalar.copy(
                            sch[:, :, 1],
                            pk[hb][1][:].rearrange("p (w9 c) -> p w9 c",
                                                   w9=2))

            # ---------------- projection ----------------
            SQUARE = mybir.ActivationFunctionType.Square
            for hb in range(2):
                sq = cpool.tile([128, 32], F32, tag=f"sq{hb}", name=f"sq{hb}")
                rs = cpool.tile([128, 1], F32, tag=f"rs{hb}", name=f"rs{hb}")
                nc.scalar.activation(sq[:], psK3[hb][:], SQUARE,
                                     accum_out=rs[:])
                psq = ppool.tile([16, 1], F32, tag="ps", name=f"q{hb}")
                nc.tensor.matmul(psq[:], t_pk32[:, 0:16], rs[:],
                                 start=True, stop=False)
                nc.tensor.matmul(psq[:], t_pk32[0:1, 18:34],
                                 t_pk32[0:1, 16:17], start=False, stop=True)
                res = cpool.tile([16, 1], F32, tag=f"res{hb}",
                                 name=f"res{hb}")
                nc.vector.tensor_copy(res[:], psq[:])
                nc.sync.dma_start(d_out[:, hb * HB:hb * HB + HB], res[:])

    nc.compile()
    return nc


# ---------------------------------------------------------------------------
# Entry point
# ---------------------------------------------------------------------------


def kernel(features, weights):
    global _PROGRAM
    from concourse.bass_utils import run_bass_kernel_spmd

    features = np.asarray(features)
    weights = np.asarray(weights)
    if _PROGRAM is None:
        _PROGRAM = _build_program()
    nc = _PROGRAM

    mats = _make_mats(weights)
    pk32 = _make_pk32()
    in_maps = []
    for c in range(NCORES):
        fc = features[c * B_CORE:(c + 1) * B_CORE]
        in_maps.append({
            "pk16": _make_pk16(fc, weights.astype(np.float64).reshape(DEPTH, NQ)[0]),
            "pk32": pk32,
            "mats": mats,
        })

    last_err = None
    for attempt in range(3):
        try:
            res = run_bass_kernel_spmd(nc, in_maps, list(range(NCORES)))
            break
        except Exception as e:  # noqa: BLE001
            last_err = e
            import time

            time.sleep(10 * (attempt + 1))
    else:
        raise last_err
    out = np.concatenate([res.results[c]["out"][0] for c in range(NCORES)])
    return out.astype(np.float32)


if __name__ == "__main__":
    rng = np.random.default_rng(0)
    f = rng.standard_normal((256, 10)).astype(np.float32)
    w = (0.01 * rng.random((4, 10))).astype(np.float32)
    print(kernel(f, w)[:8])


# revision 37
# speedup vs baseline: 1.0157x; 1.0157x over previous
"""Trainium2 Bass kernel for the 14-wire quantum autoencoder swap test.

Math: wires 10-13 stay |0> until the swap test, so
P(aux=1) = 0.5 - 0.5*q with q = sum_{trash wires 7,8,9 = 0} |c_i|^2 of the
10-qubit state after AngleEmbedding + BasicEntanglerLayers.

All transforms run on the PE in fp16 (fp32 PSUM accumulation), data-parallel
over 8 cores (32 samples each), two 16-sample half-batch pipelines per core:
  state S~ [ft, w] per half, ft = par*64 + w0*32 + w1*16 + bl
  (par = w2 after C01,C12; w = wires 3..9 index, w9 = MSB).
  Per layer: stage G (RX0-2 + C01 + C12; C90 of the previous layer folded in
  via row-permuted GkC90 blocks applied to the w9=1 column half) flips to
  standard layout [w, ft']; stage K (RX3-9 + C34..C89, C23 via K2/K2b
  stationary split by par class) flips back. Zero-padded stationary columns
  keep every matmul writing all 128 PSUM partitions. Negations live in
  host-built blocks: [Xim_neg | Xre | Xim] per matrix. PSUM->SBUF copies are
  split re(ACT)/im(DVE) and the per-layer emission interleaves both halves
  so the PE never stalls on one half's copies.
The embedding is folded into layer 0's G stage: S~0 = diag(F) @ P with P the
per-sample wires-3..9 seed (host-replicated over g classes) and GF =
diag(F) @ Gk0 host-premultiplied. Layer 3's K stage computes only the 16
w<16 output columns the projection needs; the finale is an ACT
square+row-accumulate, a (-0.5*sel)-matmul with +0.5 bias row, and one DMA
per half.
"""
import numpy as np

NCORES = 8
B_CORE = 32
HB = 16
DEPTH = 4
NQ = 10

# pk16 fp16 [128, 1728]: P per half (2x384, [imn|re|im]) |
# GF = diag(F)*Gk0 per half (2x384, [imn|re|im]) | zeros 192
C_P = 0        # + hb*640: [Pre 128 | Pim 128]
C_GF = 256     # + hb*640: [GFimn | GFre | GFim]
C_ZERO = 1280
PK16 = 1472

# pk32 f32 [128, 34]: cols 0:16 = -0.5*sel, col 16 = 1.0, cols 18:34 = 0.5
PK32 = 34

# mats fp16: l0: kR 384 | kB 384; l1/l2: gR | gC | kR | kB;
# l3: gR | gC | kR16 (48) | kB16 (48)
L_COLS = [768, 1536, 1536, 864]
L_OFF = [0, 768, 2304, 3840]
M_COLS = 4704

# ---------------------------------------------------------------------------
# Host-side constant construction
# ---------------------------------------------------------------------------

# ft class (par*4 + gg) -> g = w0*4 + w1*2 + w2
_FT_G = np.zeros(8, dtype=np.int64)
for _par in range(2):
    for _gg in range(4):
        _FT_G[_par * 4 + _gg] = (_gg >> 1) * 4 + (_gg & 1) * 2 + _par


def _perm_matrix(perm):
    m = np.zeros((len(perm), len(perm)))
    for src, dst in enumerate(perm):
        m[dst, src] = 1.0
    return m


def _cnot_chain_perm_p():
    perm = np.zeros(128, dtype=np.int64)
    for p in range(128):
        w = [(p >> k) & 1 for k in range(7)]
        for k in range(6):
            w[k + 1] ^= w[k]
        perm[p] = sum(w[k] << k for k in range(7))
    return perm


def _build_k2(weights_l):
    m = np.array([[1.0]], dtype=np.complex128)
    for w in (9, 8, 7, 6, 5, 4, 3):
        c, s = np.cos(weights_l[w] / 2), np.sin(weights_l[w] / 2)
        r = np.array([[c, -1j * s], [-1j * s, c]], dtype=np.complex128)
        m = np.kron(m, r)
    qa = _perm_matrix(_cnot_chain_perm_p())
    k2 = qa @ m
    k2b = k2 @ _perm_matrix(np.arange(128) ^ 1)
    return k2, k2b


def _build_gk(weights_l):
    m = np.array([[1.0]], dtype=np.complex128)
    for w in (0, 1, 2):
        c, s = np.cos(weights_l[w] / 2), np.sin(weights_l[w] / 2)
        r = np.array([[c, -1j * s], [-1j * s, c]], dtype=np.complex128)
        m = np.kron(m, r)
    perm = np.zeros(8, dtype=np.int64)
    for g in range(8):
        w0, w1, w2 = (g >> 2) & 1, (g >> 1) & 1, g & 1
        w1 ^= w0
        w2 ^= w1
        perm[g] = w0 * 4 + w1 * 2 + w2
    G = _perm_matrix(perm) @ m
    gk = np.zeros((128, 128), dtype=np.complex128)
    for ci in range(8):
        for co in range(8):
            v = G[_FT_G[co], _FT_G[ci]]
            if v != 0:
                for bl in range(HB):
                    gk[ci * 16 + bl, co * 16 + bl] = v
    return gk  # contraction: T[ft_out] = sum_ft_in S[ft_in] * gk[ft_in, ft_out]


def _blocks(m):
    """[im_neg | re | im] fp column triple for complex matrix m [128,128]."""
    return np.concatenate([-m.imag, m.real, m.imag], axis=1)


def _make_mats(weights):
    wt = weights.astype(np.float64).reshape(DEPTH, NQ)
    flip = np.arange(128) ^ 32
    cols = []
    for l in range(DEPTH):
        k2, k2b = _build_k2(wt[l])
        if l >= 1:
            gk = _build_gk(wt[l])
            cols.append(_blocks(gk))
            cols.append(_blocks(gk[flip, :]))
        if l < DEPTH - 1:
            cols.append(_blocks(k2.T))
            cols.append(_blocks(k2b.T))
        else:
            cols.append(_blocks(k2.T[:, 0:16]))
            cols.append(_blocks(k2b.T[:, 0:16]))
    mats = np.concatenate(cols, axis=1)
    assert mats.shape == (128, M_COLS)
    return mats.astype(np.float16)


def _make_pk16(features_core, weights_l0):
    th = features_core.astype(np.float64)
    B = th.shape[0]
    c_emb, s_emb = np.cos(th / 2), np.sin(th / 2)
    v = np.stack([c_emb.astype(np.complex128), -1j * s_emb], axis=-1)

    pk = np.zeros((128, PK16), dtype=np.float64)
    # seed over wires 3..9 per sample: pt[b, j], j = w9*64+...+w3
    pt = np.ones((B, 128), dtype=np.complex128)
    for j in range(128):
        val = np.ones(B, dtype=np.complex128)
        for k, w in enumerate((3, 4, 5, 6, 7, 8, 9)):
            val = val * v[:, w, (j >> k) & 1]
        pt[:, j] = val
    # F per sample per class
    Fv = np.zeros((B, 8), dtype=np.complex128)
    for cls in range(8):
        g = _FT_G[cls]
        w0, w1, w2 = (g >> 2) & 1, (g >> 1) & 1, g & 1
        Fv[:, cls] = v[:, 0, w0] * v[:, 1, w1] * v[:, 2, w2]
    gk0 = _build_gk(weights_l0)
    for hb in range(2):
        # P rows: ft = cls*16+bl -> pt[hb*16+bl]; GF = diag(F)*gk0
        P = np.zeros((128, 128), dtype=np.complex128)
        GF = np.zeros((128, 128), dtype=np.complex128)
        for cls in range(8):
            for bl in range(HB):
                ft = cls * 16 + bl
                b = hb * HB + bl
                P[ft, :] = pt[b, :]
                GF[ft, :] = Fv[b, cls] * gk0[ft, :]
        c0 = C_P + hb * 640
        pk[:, c0:c0 + 128] = P.real
        pk[:, c0 + 128:c0 + 256] = P.imag
        c1 = C_GF + hb * 640
        pk[:, c1:c1 + 128] = -GF.imag
        pk[:, c1 + 128:c1 + 256] = GF.real
        pk[:, c1 + 256:c1 + 384] = GF.imag
    return pk.astype(np.float16)


def _make_pk32():
    pk = np.zeros((128, PK32), dtype=np.float32)
    for ft in range(128):
        pk[ft, ft % 16] = -0.5
    pk[:, 16] = 1.0
    pk[:, 18:34] = 0.5
    return pk


# ---------------------------------------------------------------------------
# Bass program
# ---------------------------------------------------------------------------

_PROGRAM = None


def _build_program():
    import concourse.bacc as bacc
    import concourse.mybir as mybir
    import concourse.tile as tile

    F32 = mybir.dt.float32
    F16 = mybir.dt.float16
    MULT = mybir.AluOpType.mult
    ADD = mybir.AluOpType.add

    nc = bacc.Bacc("TRN2", target_bir_lowering=False, debug=False,
                   num_devices=NCORES)

    d_pk16 = nc.dram_tensor("pk16", [128, PK16], F16, kind="ExternalInput")
    d_pk32 = nc.dram_tensor("pk32", [128, PK32], F32, kind="ExternalInput")
    d_mats = nc.dram_tensor("mats", [128, M_COLS], F16, kind="ExternalInput")
    d_out = nc.dram_tensor("out", [1, B_CORE], F32, kind="ExternalOutput")

    with tile.TileContext(nc) as tc:
        with (
            tc.tile_pool(name="const", bufs=1) as cpool,
            tc.tile_pool(name="psum", bufs=8, space="PSUM") as ppool,
        ):
            t_pk16 = cpool.tile([128, PK16], F16, tag="pk16")
            t_pk32 = cpool.tile([128, PK32], F32, tag="pk32")
            t_mats = cpool.tile([128, M_COLS], F16, tag="mats")

            # DMAs in just-in-time order, single SP queue
            nc.sync.dma_start(t_pk16[:, 0:C_ZERO], d_pk16[:, 0:C_ZERO])
            nc.sync.dma_start(t_pk16[:, C_ZERO:PK16], d_pk16[:, C_ZERO:PK16])
            nc.sync.dma_start(t_mats[:, 0:768], d_mats[:, 0:768])
            nc.sync.dma_start(t_mats[:, 768:1536], d_mats[:, 768:1536])
            nc.sync.dma_start(t_mats[:, 1536:2304], d_mats[:, 1536:2304])
            for l in range(2, DEPTH):
                sl = slice(L_OFF[l], L_OFF[l] + L_COLS[l])
                nc.sync.dma_start(t_mats[:, sl], d_mats[:, sl])
            nc.sync.dma_start(t_pk32[:], d_pk32[:])

            # PE warm-up: starts the PE ramp clock early
            t_wu = cpool.tile([128, 32], F16, tag="wu")
            nc.gpsimd.memset(t_wu[:], 0.0)
            ps_wu = ppool.tile([32, 32], F32, tag="ps", name="wu")
            for i in range(3):
                nc.tensor.matmul(ps_wu[:], t_wu[:], t_wu[:],
                                 start=True, stop=True)

            # state tiles (persistent; zero-padded layout [128, 512]:
            # data chunks at 0,128,256,384 (64 cols), Z at 64,192,320)
            def zfill(t, eng):
                dst = t[:].rearrange("p (a b) -> p a b", a=4, b=128)
                src = t_pk16[:, C_ZERO:C_ZERO + 192].rearrange(
                    "p (a b) -> p a b", a=3, b=64)
                eng(dst[:, 0:3, 64:128], src)

            sAB = []
            tt = []
            for hb in range(2):
                a = cpool.tile([128, 512], F16, tag=f"sA{hb}", name=f"sA{hb}")
                b = cpool.tile([128, 512], F16, tag=f"sB{hb}", name=f"sB{hb}")
                t = cpool.tile([128, 512], F16, tag=f"tt{hb}", name=f"tt{hb}")
                sAB.append([a, b])
                tt.append(t)
            for hb in range(2):
                zfill(tt[hb], nc.vector.tensor_copy)
            for hb in range(2):
                zfill(sAB[hb][0], nc.vector.tensor_copy)
            for hb in range(2):
                zfill(sAB[hb][1], nc.gpsimd.tensor_copy)

            # chunk views of a zero-padded tile: [p, x(par/w9), y(ri), 64]
            def chunks(t):
                return t[:].rearrange("p (x y b) -> p x y b",
                                      x=2, y=2, b=128)[:, :, :, 0:64]

            # ---------------- layers ----------------
            def mat(c0, c1):
                return t_mats[:, c0:c1]

            psK3 = [None, None]

            def emit_g(l, hb, gR, gC):
                pgr = ppool.tile([128, 128], F32, tag="ps",
                                 name=f"pgr{l}{hb}")
                pgi = ppool.tile([128, 128], F32, tag="ps",
                                 name=f"pgi{l}{hb}")
                if l == 0:
                    c0 = C_P + hb * 640
                    c1 = C_GF + hb * 640
                    pre = t_pk16[:, c0:c0 + 128]
                    pim = t_pk16[:, c0 + 128:c0 + 256]
                    gfimn = t_pk16[:, c1:c1 + 128]
                    gfre = t_pk16[:, c1 + 128:c1 + 256]
                    gfim = t_pk16[:, c1 + 256:c1 + 384]
                    nc.tensor.matmul(pgr[:], pre, gfre,
                                     start=True, stop=False)
                    nc.tensor.matmul(pgr[:], pim, gfimn,
                                     start=False, stop=True)
                    nc.tensor.matmul(pgi[:], pre, gfim,
                                     start=True, stop=False)
                    nc.tensor.matmul(pgi[:], pim, gfre,
                                     start=False, stop=True)
                else:
                    sv = sAB[hb][(l - 1) % 2]
                    nc.tensor.matmul(pgr[:], sv[:, 0:128],
                                     mat(gR + 128, gR + 256),
                                     start=True, stop=False)
                    nc.tensor.matmul(pgr[:], sv[:, 192:320],
                                     mat(gC + 128, gC + 256),
                                     start=False, stop=False)
                    nc.tensor.matmul(pgr[:], sv[:, 128:256],
                                     mat(gR, gR + 128),
                                     start=False, stop=False)
                    nc.tensor.matmul(pgr[:], sv[:, 320:448],
                                     mat(gC, gC + 128),
                                     start=False, stop=True)
                    nc.tensor.matmul(pgi[:], sv[:, 0:128],
                                     mat(gR + 256, gR + 384),
                                     start=True, stop=False)
                    nc.tensor.matmul(pgi[:], sv[:, 192:320],
                                     mat(gC + 256, gC + 384),
                                     start=False, stop=False)
                    nc.tensor.matmul(pgi[:], sv[:, 128:256],
                                     mat(gR + 128, gR + 256),
                                     start=False, stop=False)
                    nc.tensor.matmul(pgi[:], sv[:, 320:448],
                                     mat(gC + 128, gC + 256),
                                     start=False, stop=True)
                return pgr, pgi

            def emit_k(l, hb, kR, kB):
                t = tt[hb]
                if l < DEPTH - 1:
                    pkr = ppool.tile([128, 128], F32, tag="ps",
                                     name=f"pkr{l}{hb}")
                    pki = ppool.tile([128, 128], F32, tag="ps",
                                     name=f"pki{l}{hb}")
                    nc.tensor.matmul(pkr[:], t[:, 0:128],
                                     mat(kR + 128, kR + 256),
                                     start=True, stop=False)
                    nc.tensor.matmul(pkr[:], t[:, 192:320],
                                     mat(kB + 128, kB + 256),
                                     start=False, stop=False)
                    nc.tensor.matmul(pkr[:], t[:, 128:256],
                                     mat(kR, kR + 128),
                                     start=False, stop=False)
                    nc.tensor.matmul(pkr[:], t[:, 320:448],
                                     mat(kB, kB + 128),
                                     start=False, stop=True)
                    nc.tensor.matmul(pki[:], t[:, 0:128],
                                     mat(kR + 256, kR + 384),
                                     start=True, stop=False)
                    nc.tensor.matmul(pki[:], t[:, 192:320],
                                     mat(kB + 256, kB + 384),
                                     start=False, stop=False)
                    nc.tensor.matmul(pki[:], t[:, 128:256],
                                     mat(kR + 128, kR + 256),
                                     start=False, stop=False)
                    nc.tensor.matmul(pki[:], t[:, 320:448],
                                     mat(kB + 128, kB + 256),
                                     start=False, stop=True)
                    return pkr, pki
                psK3[hb] = ppool.tile([128, 32], F32, tag="ps",
                                      name=f"pk3{hb}")
                nc.tensor.matmul(psK3[hb][:], t[:, 0:128],
                                 mat(kR + 16, kR + 48),
                                 start=True, stop=False)
                nc.tensor.matmul(psK3[hb][:], t[:, 192:320],
                                 mat(kB + 16, kB + 48),
                                 start=False, stop=False)
                nc.tensor.matmul(psK3[hb][:], t[:, 128:256],
                                 mat(kR, kR + 32),
                                 start=False, stop=False)
                nc.tensor.matmul(psK3[hb][:], t[:, 320:448],
                                 mat(kB, kB + 32),
                                 start=False, stop=True)
                return None

            for l in range(DEPTH):
                base = L_OFF[l]
                gR = base
                gC = base + 384
                kR = base + (768 if l >= 1 else 0)
                kB = kR + (48 if l == DEPTH - 1 else 384)
                pg = [emit_g(l, hb, gR, gC) for hb in range(2)]
                for hb in range(2):
                    tch = chunks(tt[hb])
                    nc.scalar.copy(
                        tch[:, :, 0],
                        pg[hb][0][:].rearrange("p (par c) -> p par c", par=2))
                    nc.vector.tensor_copy(
                        tch[:, :, 1],
                        pg[hb][1][:].rearrange("p (par c) -> p par c", par=2))
                pk = [emit_k(l, hb, kR, kB) for hb in range(2)]
                if l < DEPTH - 1:
                    for hb in range(2):
                        sch = chunks(sAB[hb][l % 2])
                        nc.scalar.copy(
                            sch[:, :, 0],
                            pk[hb][0][:].rearrange("p (w9 c) -> p w9 c",
                                                   w9=2))
                        nc.vector.tensor_copy(
                            sch[:, :, 1],
                            pk[hb][1][:].rearrange("p (w9 c) -> p w9 c",
                                                   w9=2))

            # ---------------- projection ----------------
            SQUARE = mybir.ActivationFunctionType.Square
            for hb in range(2):
                sq = cpool.tile([128, 32], F32, tag=f"sq{hb}", name=f"sq{hb}")
                rs = cpool.tile([128, 1], F32, tag=f"rs{hb}", name=f"rs{hb}")
                nc.scalar.activation(sq[:], psK3[hb][:], SQUARE,
                                     accum_out=rs[:])
                psq = ppool.tile([16, 1], F32, tag="ps", name=f"q{hb}")
                nc.tensor.matmul(psq[:], t_pk32[:, 0:16], rs[:],
                                 start=True, stop=False)
                nc.tensor.matmul(psq[:], t_pk32[0:1, 18:34],
                                 t_pk32[0:1, 16:17], start=False, stop=True)
                res = cpool.tile([16, 1], F32, tag=f"res{hb}",
                                 name=f"res{hb}")
                nc.vector.tensor_copy(res[:], psq[:])
                nc.sync.dma_start(d_out[:, hb * HB:hb * HB + HB], res[:])

    nc.compile()
    return nc


# ---------------------------------------------------------------------------
# Entry point
# ---------------------------------------------------------------------------


def kernel(features, weights):
    global _PROGRAM
    from concourse.bass_utils import run_bass_kernel_spmd

    features = np.asarray(features)
    weights = np.asarray(weights)
    if _PROGRAM is None:
        _PROGRAM = _build_program()
    nc = _PROGRAM

    mats = _make_mats(weights)
    pk32 = _make_pk32()
    in_maps = []
    for c in range(NCORES):
        fc = features[c * B_CORE:(c + 1) * B_CORE]
        in_maps.append({
            "pk16": _make_pk16(fc, weights.astype(np.float64).reshape(DEPTH, NQ)[0]),
            "pk32": pk32,
            "mats": mats,
        })

    last_err = None
    for attempt in range(3):
        try:
            res = run_bass_kernel_spmd(nc, in_maps, list(range(NCORES)))
            break
        except Exception as e:  # noqa: BLE001
            last_err = e
            import time

            time.sleep(10 * (attempt + 1))
    else:
        raise last_err
    out = np.concatenate([res.results[c]["out"][0] for c in range(NCORES)])
    return out.astype(np.float32)


if __name__ == "__main__":
    rng = np.random.default_rng(0)
    f = rng.standard_normal((256, 10)).astype(np.float32)
    w = (0.01 * rng.random((4, 10))).astype(np.float32)
    print(kernel(f, w)[:8])
